# revision 13
# baseline (speedup 1.0000x reference)
import sys
import functools

sys.path.insert(0, "/opt/trn_rl_repo")
import numpy as np
import ml_dtypes

# Problem constants (nn_Causal_GraphConvolution): hardcoded per harness contract.
K = 2
N = 8192
IN_F = 128
OUT_F = 64
NCORES = 8
ROWS = N // NCORES   # 1024 attention rows per core
NCH = N // 128       # 64 column chunks of 128
CPG = 4              # adjacency chunks per DMA group
NGRP = NCH // CPG    # 16 adjacency tiles

ADJ_FP8 = True       # adjacency stored fp8 in DRAM, cast to bf16 on DMA

# Row-rescaled attention: softmax rows are scale-invariant, so instead of
# p = adj * max(u*v, 1) with u = exp(Wh1[r]) we use p = adj * max(v, w)
# with v = exp(Wh2[m]), w = exp(-Wh1[r]).  Three equivalent per-group paths:
#   D: DVE tensor_scalar max (4x) + batched DVE tensor_tensor mask (2x)
#   A: ACT relu(w - v) + DVE mask + PE correction adj @ (v*Whp)
#   G: DVE tensor_scalar max build + batched GPSIMD tensor_tensor mask
# Static per-(k, group) assignment; k0's GPSIMD groups sit late so they
# don't contend with the SWDGE adjacency-stream descriptor generation.
PATH_A_K = {0: (3, 6, 8, 11, 13), 1: (3, 6, 8, 12, 14)}
PATH_G_K = {0: (10, 12, 14), 1: (1, 5, 10)}


@functools.lru_cache(maxsize=3)
def _build():
    import concourse.bacc as bacc
    import concourse.tile as tile
    from concourse.tile import add_dep_helper
    from concourse import mybir

    bf16 = mybir.dt.bfloat16
    f32 = mybir.dt.float32
    f8e4 = mybir.dt.float8e4
    AO = mybir.AluOpType
    AF = mybir.ActivationFunctionType
    adj_dt = f8e4 if ADJ_FP8 else bf16

    nc = bacc.Bacc(num_devices=NCORES)

    # Per-core inputs (the SPMD in_maps supply different data per core).
    # adjT[g, p, j, r] = adj[core_rows[r], (g*CPG+j)*128 + p]
    adjT = nc.declare_dram_parameter("adjT", [NGRP, 128, CPG, ROWS], adj_dt, False)
    xT = nc.declare_dram_parameter("xT", [K, IN_F, N], bf16, False)
    xrT = nc.declare_dram_parameter("xrT", [K, IN_F, ROWS], bf16, False)
    waug = nc.declare_dram_parameter("waug", [IN_F, 66], bf16, False)
    # out[(k*64+o), r] = relu(adj @ h')[core_rows[r], k, o] (transposed layout)
    out = nc.declare_dram_parameter("out", [K * OUT_F, ROWS], f32, True)

    wrow = nc.dram_tensor("wrow", [K, 1, ROWS], bf16)
    hp_local = [nc.dram_tensor(f"hp_local{k}", [ROWS, OUT_F], bf16) for k in range(K)]
    hp_full = [
        nc.dram_tensor(f"hp_full{k}", [N, OUT_F], bf16, addr_space="Shared")
        for k in range(K)
    ]

    paths = {}
    for k in range(K):
        for g in range(NGRP):
            paths[(k, g)] = "D"
        for g in PATH_A_K[k]:
            paths[(k, g)] = "A"
        for g in PATH_G_K[k]:
            paths[(k, g)] = "G"
        assert paths[(k, 0)] == "D" and paths[(k, NGRP - 1)] == "D"

    XE = N // 8  # x eighth-chunk width (1024)

    with tile.TileContext(nc) as tc:
        with (
            tc.tile_pool(name="persist", bufs=1) as persist,
            tc.tile_pool(name="adjp", bufs=NGRP) as adjp,
            tc.tile_pool(name="xp", bufs=1) as xp,
            tc.tile_pool(name="tp", bufs=2) as tp,
            tc.tile_pool(name="pp", bufs=2) as pp,
            tc.tile_pool(name="whvp", bufs=4) as whvp,
            tc.tile_pool(name="fin", bufs=1) as fin,
            tc.tile_pool(name="hpio", bufs=4) as hpio,
            tc.tile_pool(name="sm", bufs=2) as sm,
        ):
            # ---- PSUM: 4 accT banks (phase 1) + [2 ps0 + 1 psu] banks
            # (phase 0, freed for the 2 outT banks of phase 2) ----
            psmain_cm = tc.tile_pool(name="psmain", bufs=1, space="PSUM")
            psmain = psmain_cm.__enter__()
            accT = {
                k: [
                    psmain.tile([65, 512], f32, tag=f"accT{k}_{h}",
                                name=f"accT{k}_{h}")
                    for h in range(2)
                ]
                for k in range(K)
            }
            psAB_cm = tc.tile_pool(name="psAB", bufs=1, space="PSUM")
            psAB = psAB_cm.__enter__()

            # phase-0 inputs first so they are not queued behind the 8MB
            # adjacency stream
            waug_sb = persist.tile([IN_F, 66], bf16, tag="waug")
            nc.gpsimd.dma_start(out=waug_sb, in_=waug[:])
            xbigs, xrs = [], []
            x_dmas = []
            for k in range(K):
                xr_sb = xp.tile([128, ROWS], bf16, tag="xr", bufs=2, name=f"xr{k}")
                nc.gpsimd.dma_start(out=xr_sb, in_=xrT[k])
                xrs.append(xr_sb)
                xb_k = []
                for e in range(8):
                    xbig = xp.tile([128, XE], bf16, tag="xbig", bufs=2,
                                   name=f"xbig{k}_{e}")
                    x_dmas.append(nc.sync.dma_start(
                        out=xbig, in_=xT[k, :, e * XE:(e + 1) * XE]
                    ))
                    xb_k.append(xbig)
                xbigs.append(xb_k)

            # ---- resident adjacency (mask in phase 1, weights in phase 2) ----
            # fp8 in DRAM (binary 0/1, exact); SWDGE casts to bf16 on the way
            # into SBUF. Chain group loads (2 in flight) to land in order.
            adj_sb = []
            adj_dmas = []
            for g in range(NGRP):
                at = adjp.tile([128, CPG, ROWS], bf16, tag="adjt", name=f"adj{g}")
                if ADJ_FP8:
                    d = nc.gpsimd.dma_start(out=at, in_=adjT[g])
                else:
                    d = nc.sync.dma_start(out=at, in_=adjT[g])
                if g >= 2:
                    add_dep_helper(d.ins, adj_dmas[g - 2].ins,
                                   reason="stream adjacency in order")
                else:
                    add_dep_helper(d.ins, x_dmas[7].ins,
                                   reason="k0 x lands before adjacency stream")
                adj_dmas.append(d)
                adj_sb.append(at)

            # ---- phase 0 per k: w_bc, whp (= [Wh | 1]), v, v_neg ----
            whp, v_sb, v_neg, w_bc = [], [], [], []
            for k in range(K):
                # w = exp(-Wh1[core rows]), broadcast along partitions
                for half in range(2):
                    psu = psAB.tile([1, 512], f32, tag="psu", name=f"psu{k}_{half}")
                    nc.tensor.matmul(
                        psu,
                        lhsT=waug_sb[:, 64:65],
                        rhs=xrs[k][:, half * 512:(half + 1) * 512],
                        start=True,
                        stop=True,
                    )
                    uh = sm.tile([1, 512], bf16, tag="uh", name=f"uh{k}_{half}")
                    nc.scalar.activation(uh, psu, AF.Exp, scale=-1.0)
                    nc.gpsimd.dma_start(
                        out=wrow[k, :, half * 512:(half + 1) * 512], in_=uh
                    )
                wb = persist.tile([128, ROWS], bf16, tag=f"wb{k}")
                nc.gpsimd.dma_start(out=wb, in_=wrow[k].to_broadcast((128, ROWS)))
                w_bc.append(wb)

                # Wh_aug = x @ [W | W@a1 | W@a2]
                whp_k = persist.tile([128, NCH, 65], bf16, tag=f"whp{k}")
                nc.vector.memset(whp_k[:, :, 64:65], 1.0)
                wh2_k = persist.tile([128, NCH], f32, tag=f"wh2{k}")
                for e in range(8):
                    xbig = xbigs[k][e]
                    for cb in range(2):
                        ps0 = psAB.tile([128, 4, 66], f32, tag="ps0",
                                        bufs=2, name=f"ps0_{k}_{e}_{cb}")
                        for j in range(4):
                            ch = cb * 4 + j
                            nc.tensor.matmul(
                                ps0[:, j, :],
                                lhsT=xbig[:, ch * 128:(ch + 1) * 128],
                                rhs=waug_sb,
                                start=True,
                                stop=True,
                            )
                        gch = e * 8 + cb * 4
                        nc.scalar.copy(whp_k[:, gch:gch + 4, 0:64],
                                       ps0[:, :, 0:64])
                        nc.vector.tensor_copy(wh2_k[:, gch:gch + 4],
                                              ps0[:, :, 65])
                whp.append(whp_k)

                # v = exp(Wh2), split so phase 1 unblocks after first half
                v_k = persist.tile([128, NCH], f32, tag=f"v{k}")
                nc.scalar.activation(v_k[:, 0:NCH // 2], wh2_k[:, 0:NCH // 2],
                                     AF.Exp)
                nc.scalar.activation(v_k[:, NCH // 2:], wh2_k[:, NCH // 2:],
                                     AF.Exp)
                v_sb.append(v_k)
                vn_k = persist.tile([128, NCH], f32, tag=f"vn{k}")
                nc.vector.tensor_scalar_mul(vn_k[:, 0:NCH // 2],
                                            v_k[:, 0:NCH // 2], -1.0)
                nc.vector.tensor_scalar_mul(vn_k[:, NCH // 2:],
                                            v_k[:, NCH // 2:], -1.0)
                v_neg.append(vn_k)

            # psAB stays open through phase B emission; its banks are only
            # reused by outT (phase 2), which runs long after phase 0.

            # ---- phase 1 for one k, one adjacency group ----
            # p[m, r] = adj[m, r] * max(v[m], w[r]); accumulate
            # accT[k][h][o, r] += whp_ch[m, o]^T @ p[m, r] over all 64 chunks.
            def phase1_group(k, g):
                path = paths[(k, g)]
                p4 = pp.tile([128, CPG, ROWS], bf16, tag="p", name=f"p{k}_{g}")
                t4 = tp.tile([128, CPG, ROWS], bf16, tag="t", name=f"t{k}_{g}")
                for j in range(CPG):
                    ch = g * CPG + j
                    if path == "A":
                        nc.scalar.activation(
                            t4[:, j, :], w_bc[k], AF.Relu,
                            bias=v_neg[k][:, ch:ch + 1], scale=1.0,
                        )
                    else:
                        nc.vector.tensor_scalar(
                            out=t4[:, j, :],
                            in0=w_bc[k],
                            scalar1=v_sb[k][:, ch:ch + 1],
                            scalar2=None,
                            op0=AO.max,
                        )
                if path == "G":
                    nc.gpsimd.tensor_mul(p4, t4, adj_sb[g])
                else:
                    nc.vector.tensor_mul(p4, t4, adj_sb[g])
                for j in range(CPG):
                    ch = g * CPG + j
                    whv = None
                    if path == "A":
                        whv = whvp.tile([128, 65], bf16, tag="whv",
                                        name=f"whv{k}_{ch}")
                        nc.vector.tensor_scalar_mul(
                            whv, whp[k][:, ch, :], v_sb[k][:, ch:ch + 1]
                        )
                    for h in range(2):
                        nc.tensor.matmul(
                            accT[k][h],
                            lhsT=whp[k][:, ch, :],
                            rhs=p4[:, j, h * 512:(h + 1) * 512],
                            start=(ch == 0),
                            stop=(ch == NCH - 1),
                        )
                        if whv is not None:
                            nc.tensor.matmul(
                                accT[k][h],
                                lhsT=whv,
                                rhs=adj_sb[g][:, j, h * 512:(h + 1) * 512],
                                start=False,
                                stop=False,
                            )

            # ---- phase 1 finish: normalize h' at source, gather ----
            # accT rows 0..63 = h' numerator (transposed), row 64 = denominator.
            # Transpose 128-col blocks back to [m, o] layout via the DMA xbar,
            # then the reciprocal lands in partition layout (cheap on DVE).
            t_engines = [nc.sync, nc.scalar]

            def phase1_finish(k):
                hpT = fin.tile([80, ROWS], bf16, tag=f"hpT{k}")
                nc.vector.memset(hpT[64:80, :], 0.0)
                for h in range(2):
                    nc.scalar.copy(hpT[0:65, h * 512:(h + 1) * 512],
                                   accT[k][h])
                hp_acc = fin.tile([128, 8, 80], bf16, tag=f"hpacc{k}")
                for b in range(8):
                    t_engines[b % 2].dma_start_transpose(
                        out=hp_acc[:, b, :],
                        in_=hpT[:, b * 128:(b + 1) * 128],
                    )
                rs = sm.tile([128, 8], f32, tag="rs", name=f"rs{k}")
                nc.vector.reciprocal(rs, hp_acc[:, :, 64])
                hp_norm = fin.tile([128, 8, OUT_F], bf16, tag=f"hpn{k}")
                for b in range(8):
                    nc.vector.tensor_scalar_mul(
                        hp_norm[:, b, :], hp_acc[:, b, 0:OUT_F], rs[:, b:b + 1]
                    )
                nc.sync.dma_start(
                    out=hp_local[k][:].rearrange("(b p) o -> p b o", p=128),
                    in_=hp_norm,
                )
                nc.gpsimd.collective_compute(
                    "AllGather",
                    mybir.AluOpType.bypass,
                    replica_groups=[list(range(NCORES))],
                    ins=[hp_local[k][:]],
                    outs=[hp_full[k][:]],
                )

            # ---- schedule phase 1 ----
            for k in range(K):
                for g in range(NGRP):
                    phase1_group(k, g)
                phase1_finish(k)

            # ---- phase 2 (k-merged): outT[(k,o), r] += hp[m,(k,o)]^T @ adjT ----
            psAB_cm.__exit__(None, None, None)
            psO_cm = tc.tile_pool(name="psO", bufs=1, space="PSUM")
            psO = psO_cm.__enter__()
            outT = [
                psO.tile([128, 512], f32, tag=f"outT{h}", name=f"outT{h}")
                for h in range(2)
            ]
            for ch in range(NCH):
                g, j = ch // CPG, ch % CPG
                hpbig = hpio.tile([128, K * OUT_F], bf16, tag="hpbig",
                                  name=f"hpbig{ch}")
                for k in range(K):
                    nc.sync.dma_start(
                        out=hpbig[:, k * OUT_F:(k + 1) * OUT_F],
                        in_=hp_full[k][ch * 128:(ch + 1) * 128, :],
                    )
                for h in range(2):
                    nc.tensor.matmul(
                        outT[h],
                        lhsT=hpbig,
                        rhs=adj_sb[g][:, j, h * 512:(h + 1) * 512],
                        start=(ch == 0),
                        stop=(ch == NCH - 1),
                    )

            out_sb = hpio.tile([128, ROWS], f32, tag="outsb", bufs=1)
            for h in range(2):
                if h == 0:
                    nc.vector.tensor_scalar_max(
                        out_sb[:, h * 512:(h + 1) * 512], outT[h], 0.0
                    )
                else:
                    nc.scalar.activation(
                        out_sb[:, h * 512:(h + 1) * 512], outT[h], AF.Relu
                    )
            nc.sync.dma_start(out=out[:], in_=out_sb)
            psO_cm.__exit__(None, None, None)
            psmain_cm.__exit__(None, None, None)

    nc.finalize()
    return nc


def _prep_inputs(x, adj, weight, a):
    bf = ml_dtypes.bfloat16
    adj_np_dt = ml_dtypes.float8_e4m3 if ADJ_FP8 else bf
    w32 = weight.astype(np.float32)
    a32 = a.astype(np.float32)
    waug = np.concatenate(
        [w32, w32 @ a32[:OUT_F], w32 @ a32[OUT_F:]], axis=1
    ).astype(bf)  # [128, 66]
    xT = np.ascontiguousarray(x.astype(np.float32).transpose(0, 2, 1)).astype(bf)
    adj_t = adj.astype(adj_np_dt)
    in_maps = []
    for c in range(NCORES):
        rows = slice(c * ROWS, (c + 1) * ROWS)
        # [N, ROWS] -> [NGRP, 128, CPG, ROWS]; chunk ch = g*CPG+j sits at
        # adj^T rows ch*128 ... ch*128+128
        adjT_c = (
            np.ascontiguousarray(adj_t[rows].T)
            .reshape(NGRP, CPG, 128, ROWS)
            .transpose(0, 2, 1, 3)
        )
        adjT_c = np.ascontiguousarray(adjT_c)
        xrT_c = np.ascontiguousarray(xT[:, :, rows])
        in_maps.append({"adjT": adjT_c, "xT": xT, "xrT": xrT_c, "waug": waug})
    return in_maps


def _run(in_maps, trace=False, **kw):
    from concourse.bass_utils import run_bass_kernel_spmd

    nc = _build()
    return run_bass_kernel_spmd(nc, in_maps, list(range(NCORES)), trace=trace, **kw)


def kernel(**inputs):
    x = np.asarray(inputs["x"])
    adj = np.asarray(inputs["adj"])
    weight = np.asarray(inputs["weight"])
    a = np.asarray(inputs["a"])
    in_maps = _prep_inputs(x, adj, weight, a)
    res = _run(in_maps)
    stacked = np.stack(
        [np.asarray(res.results[c]["out"]) for c in range(NCORES)], axis=0
    )  # [NCORES, K*OUT_F, ROWS]
    full = (
        stacked.reshape(NCORES, K, OUT_F, ROWS)
        .transpose(1, 0, 3, 2)
        .reshape(K, N, OUT_F)
    )
    return np.ascontiguousarray(full).astype(np.float32)


# revision 14
# speedup vs baseline: 1.1200x; 1.1200x over previous
import sys
import functools

sys.path.insert(0, "/opt/trn_rl_repo")
import numpy as np
import ml_dtypes

# Problem constants (nn_Causal_GraphConvolution): hardcoded per harness contract.
K = 2
N = 8192
IN_F = 128
OUT_F = 64
NCORES = 8
ROWS = N // NCORES   # 1024 attention rows per core
NCH = N // 128       # 64 column chunks of 128
CPG = 4              # adjacency chunks per DMA group
NGRP = NCH // CPG    # 16 adjacency tiles

ADJ_FP8 = True       # adjacency stored fp8 in DRAM, cast to bf16 on DMA
HP_FP8 = True        # h' gathered in fp8 (halves the collective)

# Attention weights p = adj * max(u*v, 1) with u = exp(Wh1[r]), v = exp(Wh2[m])
# (exp(relu(z)) = max(exp(z), 1)).  Per-group build/mask paths:
#   D: DVE fused tensor_scalar (u*v then max 1, 4x) + DVE batched mask (2x)
#   A: ACT relu(u*v - 1) + DVE mask + PE correction adj @ Whp (same stationary)
#   G: DVE fused tensor_scalar build + batched GPSIMD tensor_tensor mask
# k0's GPSIMD groups sit late so they don't contend with the SWDGE
# adjacency-stream descriptor generation.
PATH_A_K = {0: (2, 3, 6, 8, 11, 13), 1: (2, 3, 6, 8, 12, 14)}
PATH_G_K = {0: (10, 12, 14), 1: (1, 5, 10)}


@functools.lru_cache(maxsize=3)
def _build():
    import concourse.bacc as bacc
    import concourse.tile as tile
    from concourse.tile import add_dep_helper
    from concourse import mybir

    bf16 = mybir.dt.bfloat16
    f32 = mybir.dt.float32
    f8e4 = mybir.dt.float8e4
    AO = mybir.AluOpType
    AF = mybir.ActivationFunctionType
    adj_dt = f8e4 if ADJ_FP8 else bf16
    hp_dt = f8e4 if HP_FP8 else bf16

    nc = bacc.Bacc(num_devices=NCORES)

    # Per-core inputs (the SPMD in_maps supply different data per core).
    # adjT[g, p, j, r] = adj[core_rows[r], (g*CPG+j)*128 + p]
    adjT = nc.declare_dram_parameter("adjT", [NGRP, 128, CPG, ROWS], adj_dt, False)
    xT = nc.declare_dram_parameter("xT", [K, IN_F, N], bf16, False)
    xrT = nc.declare_dram_parameter("xrT", [K, IN_F, ROWS], bf16, False)
    waug = nc.declare_dram_parameter("waug", [IN_F, 66], bf16, False)
    # out[(k*64+o), r] = relu(adj @ h')[core_rows[r], k, o] (transposed layout)
    out = nc.declare_dram_parameter("out", [K * OUT_F, ROWS], f32, True)

    urow = nc.dram_tensor("urow", [K, 1, ROWS], bf16)
    hp_local = [nc.dram_tensor(f"hp_local{k}", [ROWS, OUT_F], hp_dt) for k in range(K)]
    hp_full = [
        nc.dram_tensor(f"hp_full{k}", [N, OUT_F], hp_dt, addr_space="Shared")
        for k in range(K)
    ]

    paths = {}
    for k in range(K):
        for g in range(NGRP):
            paths[(k, g)] = "D"
        for g in PATH_A_K[k]:
            paths[(k, g)] = "A"
        for g in PATH_G_K[k]:
            paths[(k, g)] = "G"
        assert paths[(k, 0)] == "D" and paths[(k, NGRP - 1)] == "D"

    XS = 512  # x sixteenth-chunk width

    with tile.TileContext(nc) as tc:
        with (
            tc.tile_pool(name="persist", bufs=1) as persist,
            tc.tile_pool(name="adjp", bufs=NGRP) as adjp,
            tc.tile_pool(name="xp", bufs=1) as xp,
            tc.tile_pool(name="tp", bufs=2) as tp,
            tc.tile_pool(name="pp", bufs=2) as pp,
            tc.tile_pool(name="fin", bufs=1) as fin,
            tc.tile_pool(name="hpio", bufs=2) as hpio,
            tc.tile_pool(name="sm", bufs=2) as sm,
        ):
            # ---- PSUM: 4 accT banks (phase 1) + [2 ps0 + 1 psu] banks
            # (phase 0; freed for the 2 outT banks of phase 2) ----
            psmain_cm = tc.tile_pool(name="psmain", bufs=1, space="PSUM")
            psmain = psmain_cm.__enter__()
            accT = {
                k: [
                    psmain.tile([65, 512], f32, tag=f"accT{k}_{h}",
                                name=f"accT{k}_{h}")
                    for h in range(2)
                ]
                for k in range(K)
            }
            psAB_cm = tc.tile_pool(name="psAB", bufs=1, space="PSUM")
            psAB = psAB_cm.__enter__()

            # phase-0 inputs first so they are not queued behind the 8MB
            # adjacency stream
            waug_sb = persist.tile([IN_F, 66], bf16, tag="waug")
            nc.gpsimd.dma_start(out=waug_sb, in_=waug[:])
            neg1 = persist.tile([128, 1], f32, tag="neg1")
            nc.vector.memset(neg1, -1.0)
            xbigs, xrs = [], []
            x_dmas = []
            for k in range(K):
                xr_sb = xp.tile([128, ROWS], bf16, tag="xr", bufs=2, name=f"xr{k}")
                nc.gpsimd.dma_start(out=xr_sb, in_=xrT[k])
                xrs.append(xr_sb)
                xb_k = []
                for e in range(16):
                    xbig = xp.tile([128, XS], bf16, tag="xbig", bufs=2,
                                   name=f"xbig{k}_{e}")
                    x_dmas.append(nc.sync.dma_start(
                        out=xbig, in_=xT[k, :, e * XS:(e + 1) * XS]
                    ))
                    xb_k.append(xbig)
                xbigs.append(xb_k)

            # ---- resident adjacency (mask in phase 1, weights in phase 2) ----
            # fp8 in DRAM (binary 0/1, exact); SWDGE casts to bf16 on the way
            # into SBUF. Chain group loads (2 in flight) to land in order.
            adj_sb = []
            adj_dmas = []
            for g in range(NGRP):
                at = adjp.tile([128, CPG, ROWS], bf16, tag="adjt", name=f"adj{g}")
                if ADJ_FP8:
                    d = nc.gpsimd.dma_start(out=at, in_=adjT[g])
                else:
                    d = nc.sync.dma_start(out=at, in_=adjT[g])
                if g >= 2:
                    add_dep_helper(d.ins, adj_dmas[g - 2].ins,
                                   reason="stream adjacency in order")
                else:
                    add_dep_helper(d.ins, x_dmas[15].ins,
                                   reason="k0 x lands before adjacency stream")
                adj_dmas.append(d)
                adj_sb.append(at)

            # ---- phase 0 per k: u_bc, whp (= [Wh | 1]), v ----
            whp, v_sb, u_bc = [], [], []
            for k in range(K):
                # u = exp(Wh1[core rows]), broadcast along partitions
                for half in range(2):
                    psu = psAB.tile([1, 512], f32, tag="psu", name=f"psu{k}_{half}")
                    nc.tensor.matmul(
                        psu,
                        lhsT=waug_sb[:, 64:65],
                        rhs=xrs[k][:, half * 512:(half + 1) * 512],
                        start=True,
                        stop=True,
                    )
                    uh = sm.tile([1, 512], bf16, tag="uh", name=f"uh{k}_{half}")
                    nc.scalar.activation(uh, psu, AF.Exp)
                    nc.gpsimd.dma_start(
                        out=urow[k, :, half * 512:(half + 1) * 512], in_=uh
                    )
                ub = persist.tile([128, ROWS], bf16, tag=f"ub{k}")
                nc.gpsimd.dma_start(out=ub, in_=urow[k].to_broadcast((128, ROWS)))
                u_bc.append(ub)

                # Wh_aug = x @ [W | W@a1 | W@a2]
                whp_k = persist.tile([128, NCH, 65], bf16, tag=f"whp{k}")
                nc.vector.memset(whp_k[:, :, 64:65], 1.0)
                wh2_k = persist.tile([128, NCH], f32, tag=f"wh2{k}")
                for e in range(16):
                    xbig = xbigs[k][e]
                    ps0 = psAB.tile([128, 4, 66], f32, tag="ps0",
                                    bufs=2, name=f"ps0_{k}_{e}")
                    for j in range(4):
                        nc.tensor.matmul(
                            ps0[:, j, :],
                            lhsT=xbig[:, j * 128:(j + 1) * 128],
                            rhs=waug_sb,
                            start=True,
                            stop=True,
                        )
                    gch = e * 4
                    nc.scalar.copy(whp_k[:, gch:gch + 4, 0:64], ps0[:, :, 0:64])
                    nc.vector.tensor_copy(wh2_k[:, gch:gch + 4], ps0[:, :, 65])
                whp.append(whp_k)

                # v = exp(Wh2), split so phase 1 unblocks after first half
                v_k = persist.tile([128, NCH], f32, tag=f"v{k}")
                nc.scalar.activation(v_k[:, 0:NCH // 2], wh2_k[:, 0:NCH // 2],
                                     AF.Exp)
                nc.scalar.activation(v_k[:, NCH // 2:], wh2_k[:, NCH // 2:],
                                     AF.Exp)
                v_sb.append(v_k)

            # ---- phase 1 for one k, one adjacency group ----
            # t = max(u*v, 1) (or relu(u*v - 1) + adj correction on the ACT
            # path); p = t * adj; accumulate (transposed)
            # accT[k][h][o, r] += whp_ch[m, o]^T @ p[m, r] over all 64 chunks.
            def phase1_group(k, g):
                path = paths[(k, g)]
                p4 = pp.tile([128, CPG, ROWS], bf16, tag="p", name=f"p{k}_{g}")
                t4 = tp.tile([128, CPG, ROWS], bf16, tag="t", name=f"t{k}_{g}")
                for j in range(CPG):
                    ch = g * CPG + j
                    if path == "A":
                        nc.scalar.activation(
                            t4[:, j, :], u_bc[k], AF.Relu,
                            bias=neg1, scale=v_sb[k][:, ch:ch + 1],
                        )
                    else:
                        nc.vector.tensor_scalar(
                            out=t4[:, j, :],
                            in0=u_bc[k],
                            scalar1=v_sb[k][:, ch:ch + 1],
                            scalar2=1.0,
                            op0=AO.mult,
                            op1=AO.max,
                        )
                if path == "G":
                    nc.gpsimd.tensor_mul(p4, t4, adj_sb[g])
                else:
                    nc.vector.tensor_mul(p4, t4, adj_sb[g])
                for j in range(CPG):
                    ch = g * CPG + j
                    for h in range(2):
                        nc.tensor.matmul(
                            accT[k][h],
                            lhsT=whp[k][:, ch, :],
                            rhs=p4[:, j, h * 512:(h + 1) * 512],
                            start=(ch == 0),
                            stop=(ch == NCH - 1),
                        )
                        if path == "A":
                            nc.tensor.matmul(
                                accT[k][h],
                                lhsT=whp[k][:, ch, :],
                                rhs=adj_sb[g][:, j, h * 512:(h + 1) * 512],
                                start=False,
                                stop=False,
                            )

            # ---- phase 1 finish: normalize h' at source, gather ----
            # accT rows 0..63 = h' numerator (transposed), row 64 = denominator.
            # Transpose 128-col blocks back to [m, o] layout via the DMA xbar;
            # the reciprocal then lands in partition layout (cheap on DVE).
            t_engines = [nc.sync, nc.scalar]

            def phase1_finish(k):
                hpT = fin.tile([80, ROWS], bf16, tag="hpT", name=f"hpT{k}")
                nc.vector.memset(hpT[64:80, :], 0.0)
                for h in range(2):
                    nc.scalar.copy(hpT[0:65, h * 512:(h + 1) * 512], accT[k][h])
                hp_acc = fin.tile([128, 8, 80], bf16, tag="hpacc",
                                  name=f"hpacc{k}")
                for b in range(8):
                    t_engines[b % 2].dma_start_transpose(
                        out=hp_acc[:, b, :],
                        in_=hpT[:, b * 128:(b + 1) * 128],
                    )
                rs = sm.tile([128, 8], f32, tag="rs", name=f"rs{k}")
                nc.vector.reciprocal(rs, hp_acc[:, :, 64])
                hp_norm = fin.tile([128, 8, OUT_F], hp_dt, tag="hpn",
                                   name=f"hpn{k}")
                for b in range(8):
                    nc.vector.tensor_scalar_mul(
                        hp_norm[:, b, :], hp_acc[:, b, 0:OUT_F], rs[:, b:b + 1]
                    )
                nc.sync.dma_start(
                    out=hp_local[k][:].rearrange("(b p) o -> p b o", p=128),
                    in_=hp_norm,
                )
                nc.gpsimd.collective_compute(
                    "AllGather",
                    mybir.AluOpType.bypass,
                    replica_groups=[list(range(NCORES))],
                    ins=[hp_local[k][:]],
                    outs=[hp_full[k][:]],
                )

            # ---- schedule phase 1 ----
            for k in range(K):
                for g in range(NGRP):
                    phase1_group(k, g)
                phase1_finish(k)

            # ---- phase 2 (k-merged): outT[(k,o), r] += hp[m,(k,o)]^T @ adjT ----
            # hp arrives in batched 8-chunk tiles per k; a cheap interleave
            # copy builds the [128, (k0|k1)] stationary per chunk.
            psAB_cm.__exit__(None, None, None)
            psO_cm = tc.tile_pool(name="psO", bufs=1, space="PSUM")
            psO = psO_cm.__enter__()
            outT = [
                psO.tile([128, 512], f32, tag=f"outT{h}", name=f"outT{h}")
                for h in range(2)
            ]
            for g8 in range(8):
                stat8 = hpio.tile([128, 8, K * OUT_F], bf16, tag="stat8",
                                  name=f"stat8_{g8}")
                for k in range(K):
                    hpr8 = hpio.tile([128, 8, OUT_F], hp_dt, tag=f"hpr8_{k}",
                                     name=f"hpr8_{g8}_{k}")
                    nc.sync.dma_start(
                        out=hpr8,
                        in_=hp_full[k][g8 * ROWS:(g8 + 1) * ROWS, :].rearrange(
                            "(b p) o -> p b o", p=128
                        ),
                    )
                    nc.vector.tensor_copy(
                        stat8[:, :, k * OUT_F:(k + 1) * OUT_F], hpr8
                    )
                for b in range(8):
                    ch = g8 * 8 + b
                    g, j = ch // CPG, ch % CPG
                    for h in range(2):
                        nc.tensor.matmul(
                            outT[h],
                            lhsT=stat8[:, b, :],
                            rhs=adj_sb[g][:, j, h * 512:(h + 1) * 512],
                            start=(ch == 0),
                            stop=(ch == NCH - 1),
                        )

            out_sb = hpio.tile([128, ROWS], f32, tag="outsb", bufs=1)
            for h in range(2):
                if h == 0:
                    nc.vector.tensor_scalar_max(
                        out_sb[:, h * 512:(h + 1) * 512], outT[h], 0.0
                    )
                else:
                    nc.scalar.activation(
                        out_sb[:, h * 512:(h + 1) * 512], outT[h], AF.Relu
                    )
            nc.sync.dma_start(out=out[:], in_=out_sb)
            psO_cm.__exit__(None, None, None)
            psmain_cm.__exit__(None, None, None)

    nc.finalize()
    return nc


def _prep_inputs(x, adj, weight, a):
    bf = ml_dtypes.bfloat16
    adj_np_dt = ml_dtypes.float8_e4m3 if ADJ_FP8 else bf
    w32 = weight.astype(np.float32)
    a32 = a.astype(np.float32)
    waug = np.concatenate(
        [w32, w32 @ a32[:OUT_F], w32 @ a32[OUT_F:]], axis=1
    ).astype(bf)  # [128, 66]
    xT = np.ascontiguousarray(x.astype(np.float32).transpose(0, 2, 1)).astype(bf)
    adj_t = adj.astype(adj_np_dt)
    in_maps = []
    for c in range(NCORES):
        rows = slice(c * ROWS, (c + 1) * ROWS)
        # [N, ROWS] -> [NGRP, 128, CPG, ROWS]; chunk ch = g*CPG+j sits at
        # adj^T rows ch*128 ... ch*128+128
        adjT_c = (
            np.ascontiguousarray(adj_t[rows].T)
            .reshape(NGRP, CPG, 128, ROWS)
            .transpose(0, 2, 1, 3)
        )
        adjT_c = np.ascontiguousarray(adjT_c)
        xrT_c = np.ascontiguousarray(xT[:, :, rows])
        in_maps.append({"adjT": adjT_c, "xT": xT, "xrT": xrT_c, "waug": waug})
    return in_maps


def _run(in_maps, trace=False, **kw):
    from concourse.bass_utils import run_bass_kernel_spmd

    nc = _build()
    return run_bass_kernel_spmd(nc, in_maps, list(range(NCORES)), trace=trace, **kw)


def kernel(**inputs):
    x = np.asarray(inputs["x"])
    adj = np.asarray(inputs["adj"])
    weight = np.asarray(inputs["weight"])
    a = np.asarray(inputs["a"])
    in_maps = _prep_inputs(x, adj, weight, a)
    res = _run(in_maps)
    stacked = np.stack(
        [np.asarray(res.results[c]["out"]) for c in range(NCORES)], axis=0
    )  # [NCORES, K*OUT_F, ROWS]
    full = (
        stacked.reshape(NCORES, K, OUT_F, ROWS)
        .transpose(1, 0, 3, 2)
        .reshape(K, N, OUT_F)
    )
    return np.ascontiguousarray(full).astype(np.float32)
